# revision 18
# baseline (speedup 1.0000x reference)
"""Fused OOQKV attention-with-generated-transform kernel for Trainium2.

Math (per head h):
  g = gelu(x @ Wg_h + bg_h)            # [T, 64, 64] per-token transform
  q,k,v = x @ W{q,k,v}_h + b           # [T, 64]
  qg[t] = q[t] @ g[t]
  att = softmax(qg @ k^T)              # per batch, no scaling
  out_h = att @ v

Sharding: head-parallel, 1 head per core (8 heads, 8 cores).

Implementation notes (v3):
- All projections are single-pass float16 matmuls (measured on HW: every
  matmul dtype streams 1 output column/cycle at 2.4 GHz, so f16's 11-bit
  mantissa is strictly better than bf16/f32r at identical cost; fp8
  DoubleRow only packs K=256 per instruction without halving per-column
  cost, so an error-compensated fp8 scheme loses to f16 1-pass).
- Biases enter as a K=1 ones-row matmul issued FIRST into each PSUM bank
  (start=True doubles as the bank reset).
- Wg columns are host-permuted e-major (col' = e*64+d) so gelu writes
  f16 contiguously and the qg d-reduction is innermost.
- qg on DVE: one f16 tensor_tensor multiply (4x/2x fast mode) against a
  broadcast q view, then a binary tree of f16 adds down to d=8 and a
  final tensor_reduce (tensor_reduce has no DVE fast mode, so the tree
  keeps most of the reduction in fast mode).
- Phase 2 runs after phase 1 (gelu and exp live in different ACT tables;
  interleaving them costs 1.3us per table reload). Within phase 2 the
  previous batch's AV matmuls are emitted mid-stream of the next batch's
  S matmuls, baseline-style, so PE/ACT pipeline across batches.
Host divides by the softmax denominator row (v is augmented with a ones
column initialized once on-chip) and transposes during the gather.
"""

import sys

sys.path.insert(0, "/opt/trn_rl_repo")

import numpy as np

B, N, E, H, D = 4, 1024, 512, 8, 64
T = B * N                 # 4096 flattened tokens
OC = 512                  # g output chunk (one PSUM bank)
NOC = (D * D) // OC       # 8 chunks per head
NKT = E // 128            # 4 contraction k-tiles
NTT = T // 128            # 32 token tiles
QVKW = 192                # fused q|v|k projection width
M = 8                     # cores
NMT = N // 128            # m tiles per batch
NNC = N // OC             # n chunks per batch

_cache = {}


def _build():
    if "nc" in _cache:
        return _cache["nc"]
    from contextlib import ExitStack

    import concourse.bass as bass
    import concourse.bacc as bacc
    import concourse.mybir as mybir
    import concourse.tile as tile
    from concourse.masks import make_identity

    F32 = mybir.dt.float32
    F32R = mybir.dt.float32r
    F16 = mybir.dt.float16
    AF = mybir.ActivationFunctionType
    ALU = mybir.AluOpType
    AX = mybir.AxisListType

    nc = bacc.Bacc(trn_type="TRN2")

    xT_d = nc.dram_tensor("xT16", [E, T], F16, kind="ExternalInput")
    Wg_d = nc.dram_tensor("Wg16", [E, D * D], F16, kind="ExternalInput")
    bg_d = nc.dram_tensor("bg16", [1, D * D], F16, kind="ExternalInput")
    Wq_d = nc.dram_tensor("Wqvk16", [E, QVKW], F16, kind="ExternalInput")
    bq_d = nc.dram_tensor("bqvk16", [1, QVKW], F16, kind="ExternalInput")
    outT_d = nc.dram_tensor("outT", [D + 1, T], F32, kind="ExternalOutput")

    with tile.TileContext(nc) as tc, ExitStack() as ctx:
        const = ctx.enter_context(tc.tile_pool(name="const", bufs=1))
        acts = ctx.enter_context(tc.tile_pool(name="acts", bufs=1))

        xT_sb, wg_sb, wq_sb = [], [], []
        for kt in range(NKT):
            xt = const.tile([128, T], F16, tag=f"x{kt}")
            nc.sync.dma_start(xt[:], xT_d[kt * 128:(kt + 1) * 128, :])
            xT_sb.append(xt)
            wt = const.tile([128, D * D], F16, tag=f"wg{kt}")
            nc.scalar.dma_start(wt[:], Wg_d[kt * 128:(kt + 1) * 128, :])
            wg_sb.append(wt)
            qt = const.tile([128, QVKW], F16, tag=f"wq{kt}")
            nc.scalar.dma_start(qt[:], Wq_d[kt * 128:(kt + 1) * 128, :])
            wq_sb.append(qt)
        bg_sb = const.tile([1, D * D], F16)
        nc.sync.dma_start(bg_sb[:], bg_d[:, :])
        bq_sb = const.tile([1, QVKW], F16)
        nc.sync.dma_start(bq_sb[:], bq_d[:, :])

        ones32 = const.tile([1, 128], F32)
        nc.gpsimd.memset(ones32[:], 1.0)
        ones16 = const.tile([1, 128], F16)
        nc.gpsimd.tensor_copy(ones16[:], ones32[:])
        onescol = const.tile([128, 1], F32)
        nc.gpsimd.memset(onescol[:], 1.0)
        ident = const.tile([128, 128], F16)
        make_identity(nc, ident[:])

        # persistent per-head activations
        v_sb = acts.tile([128, NTT, D + 1], F32R)   # v | ones column
        ov = onescol[:]
        ones_bc = bass.AP(tensor=ov.tensor, offset=ov.offset,
                          ap=[ov.ap[0], [0, NTT]])
        vv = v_sb[:]
        vcol = bass.AP(tensor=vv.tensor, offset=vv.offset + D,
                       ap=[vv.ap[0], [D + 1, NTT]])
        nc.vector.tensor_copy(vcol, ones_bc)
        kT_sb = acts.tile([D, T], F16)
        qgT_sb = acts.tile([D, T], F16)

        espool = ctx.enter_context(tc.tile_pool(name="es", bufs=14))
        outp = ctx.enter_context(tc.tile_pool(name="outp", bufs=2))

        p1 = ExitStack()
        pmain = p1.enter_context(
            tc.tile_pool(name="pmain", bufs=5, space="PSUM"))
        ptr = p1.enter_context(
            tc.tile_pool(name="ptr", bufs=1, space="PSUM"))
        gpool = p1.enter_context(tc.tile_pool(name="gp", bufs=2))
        dpool = p1.enter_context(tc.tile_pool(name="dp", bufs=2))
        tpool = p1.enter_context(tc.tile_pool(name="tp", bufs=1))

        prev_tr = []      # (tc0, k_nat, qg_t) awaiting PE transpose

        def emit_transposes():
            for tc0_, kn, qt in prev_tr:
                for src, dst in ((kn, kT_sb), (qt, qgT_sb)):
                    p_t = ptr.tile([D, 128], F16, tag="tr", name="tr")
                    nc.tensor.transpose(p_t[:], src[:], ident[:])
                    nc.vector.tensor_copy(dst[:, tc0_:tc0_ + 128], p_t[:])
            prev_tr.clear()

        # ---------------- phase 1: projections, g, qg ----------------
        for tt in range(NTT):
            tc0 = tt * 128
            g_buf = gpool.tile([128, NOC * OC], F16, tag="g")
            pq = None
            q_t = k_nat = None

            pairs = [("qvk", 0, 1, 2, 3), (4, 5, 6, 7)]
            for pair in pairs:
                members = []
                for m_ in pair:
                    if m_ == "qvk":
                        pq = pmain.tile([128, OC], F32, tag="pg", name="pq")
                        members.append((m_, pq[:, 0:QVKW],
                                        bq_sb[:], 0, QVKW))
                    else:
                        pg = pmain.tile([128, OC], F32, tag="pg",
                                        name=f"pg{m_}")
                        members.append((m_, pg[:], bg_sb[:, m_ * OC:
                                                         (m_ + 1) * OC],
                                        m_ * OC, OC))
                # bias first: start=True resets the bank
                for m_, o, brow, c0_, w_ in members:
                    nc.tensor.matmul(o, ones16[:], brow,
                                     start=True, stop=False)
                for kt in range(NKT):
                    for m_, o, brow, c0_, w_ in members:
                        rhs = (wq_sb[kt][:] if m_ == "qvk"
                               else wg_sb[kt][:, c0_:c0_ + w_])
                        nc.tensor.matmul(
                            o, xT_sb[kt][:, tc0:tc0 + 128], rhs,
                            start=False, stop=(kt == NKT - 1))

                for m_, o, brow, c0_, w_ in members:
                    if m_ == "qvk":
                        q_t = dpool.tile([128, D], F16, tag="q")
                        nc.vector.tensor_copy(q_t[:], pq[:, 0:D])
                        k_nat = dpool.tile([128, D], F16, tag="k")
                        nc.vector.tensor_copy(k_nat[:], pq[:, 2 * D:3 * D])
                        nc.vector.tensor_copy(v_sb[:, tt, 0:D],
                                              pq[:, D:2 * D])
                    else:
                        nc.scalar.activation(
                            g_buf[:, m_ * OC:(m_ + 1) * OC], o, AF.Gelu)

            # qg contraction on DVE (all f16, fast mode)
            prod = dpool.tile([128, NOC * OC], F16, tag="prod")
            gv = g_buf[:]
            g3 = bass.AP(tensor=gv.tensor, offset=gv.offset,
                         ap=[gv.ap[0], [D, D], [1, D]])
            qv = q_t[:]
            q3 = bass.AP(tensor=qv.tensor, offset=qv.offset,
                         ap=[qv.ap[0], [0, D], [1, D]])
            pv = prod[:]
            p3 = bass.AP(tensor=pv.tensor, offset=pv.offset,
                         ap=[pv.ap[0], [D, D], [1, D]])
            nc.vector.tensor_tensor(p3, g3, q3, op=ALU.mult)

            tre1 = tpool.tile([128, D * 32], F16, tag="t1")
            tre2 = tpool.tile([128, D * 16], F16, tag="t2")
            tre3 = tpool.tile([128, D * 8], F16, tag="t3")
            lvl_in, w_ = prod, 64
            for tre in (tre1, tre2, tre3):
                h_ = w_ // 2
                iv = lvl_in[:]
                a0 = bass.AP(tensor=iv.tensor, offset=iv.offset,
                             ap=[iv.ap[0], [w_, D], [1, h_]])
                a1 = bass.AP(tensor=iv.tensor, offset=iv.offset + h_,
                             ap=[iv.ap[0], [w_, D], [1, h_]])
                ov_ = tre[:]
                o3 = bass.AP(tensor=ov_.tensor, offset=ov_.offset,
                             ap=[ov_.ap[0], [h_, D], [1, h_]])
                nc.vector.tensor_tensor(o3, a0, a1, op=ALU.add)
                lvl_in, w_ = tre, h_
            qg_t = dpool.tile([128, D], F16, tag="qg")
            t3v = tre3[:]
            t33 = bass.AP(tensor=t3v.tensor, offset=t3v.offset,
                          ap=[t3v.ap[0], [8, D], [1, 8]])
            with nc.allow_low_precision(reason="f16 qg partials"):
                nc.vector.tensor_reduce(qg_t[:], t33, axis=AX.X, op=ALU.add)

            emit_transposes()
            prev_tr.append((tc0, k_nat, qg_t))
        emit_transposes()

        # ---------------- phase 2: attention ----------------
        p1.close()
        p2 = ExitStack()
        spair = p2.enter_context(
            tc.tile_pool(name="spair", bufs=3, space="PSUM"))
        pav = p2.enter_context(
            tc.tile_pool(name="pav", bufs=2, space="PSUM"))

        pending_av = []   # (b, es-pair list) awaiting AV emission

        def emit_av():
            if not pending_av:
                return
            for bb, pairs_ in pending_av:
                for nch in range(NNC):
                    pv_ = pav.tile([D + 1, OC], F32, tag="av",
                                   name=f"pav{nch}")
                    for mt in range(NMT):
                        nc.tensor.matmul(
                            pv_[:], v_sb[:, bb * NMT + mt, :],
                            pairs_[mt][:, nch * OC:(nch + 1) * OC],
                            start=(mt == 0), stop=(mt == NMT - 1))
                    nc0 = bb * N + nch * OC
                    o_t = outp.tile([D + 1, OC], F32, tag="o", name="o_t")
                    nc.vector.tensor_copy(o_t[:], pv_[:])
                    nc.sync.dma_start(outT_d[:, nc0:nc0 + OC], o_t[:])
            pending_av.clear()

        for b in range(B):
            bpairs = []
            for mt in range(NMT):
                if mt == 2:
                    emit_av()     # prior batch's AV, mid-stream
                mc0 = b * N + mt * 128
                sp = spair.tile([128, 2 * OC], F32, tag="sp", name="sp")
                for nch in range(NNC):
                    nc0 = b * N + nch * OC
                    nc.tensor.matmul(sp[:, nch * OC:(nch + 1) * OC],
                                     kT_sb[:, mc0:mc0 + 128],
                                     qgT_sb[:, nc0:nc0 + OC],
                                     start=True, stop=True)
                e_t = espool.tile([128, 2 * OC], F32R, tag="es")
                nc.scalar.activation(e_t[:], sp[:], AF.Exp)
                bpairs.append(e_t)
            pending_av.append((b, bpairs))
        emit_av()
        p2.close()

    nc.compile()
    _cache["nc"] = nc
    return nc


def _make_in_maps(x, Wq, bq, Wk, bk, Wv, bv, Wg, bg):
    import ml_dtypes  # noqa: F401
    F16 = np.float16
    xT16 = np.ascontiguousarray(
        np.asarray(x, np.float32).reshape(T, E).T).astype(F16)

    # e-major permutation for Wg columns within each head
    new_e, new_d = np.meshgrid(np.arange(D), np.arange(D), indexing="ij")
    old_of_new = (new_d * D + new_e).reshape(-1)

    in_maps = []
    for h in range(M):
        c0 = h * D
        g0 = h * D * D
        Wp = np.asarray(Wg, np.float32)[:, g0:g0 + D * D][:, old_of_new]
        bgp = np.asarray(bg, np.float32)[g0:g0 + D * D][old_of_new]

        QW = np.zeros((E, QVKW), np.float32)
        QW[:, 0:D] = Wq[:, c0:c0 + D]
        QW[:, D:2 * D] = Wv[:, c0:c0 + D]
        QW[:, 2 * D:3 * D] = Wk[:, c0:c0 + D]
        qb = np.zeros((1, QVKW), np.float32)
        qb[0, 0:D] = bq[c0:c0 + D]
        qb[0, D:2 * D] = bv[c0:c0 + D]
        qb[0, 2 * D:3 * D] = bk[c0:c0 + D]

        in_maps.append(dict(
            xT16=xT16,
            Wg16=np.ascontiguousarray(Wp).astype(F16),
            bg16=bgp.reshape(1, D * D).astype(F16),
            Wqvk16=np.ascontiguousarray(QW).astype(F16),
            bqvk16=qb.astype(F16),
        ))
    return in_maps


def kernel(x, Wq, bq, Wk, bk, Wv, bv, Wg, bg):
    from concourse import bass_utils

    nc = _build()
    in_maps = _make_in_maps(x, Wq, bq, Wk, bk, Wv, bv, Wg, bg)
    res = bass_utils.run_bass_kernel_spmd(nc, in_maps, core_ids=list(range(M)))
    out = np.empty((B, N, H, D), dtype=np.float32)
    for h in range(M):
        oT = res.results[h]["outT"]           # [65, T]
        o = (oT[:D] / oT[D:D + 1]).T          # [T, 64]
        out[:, :, h, :] = o.reshape(B, N, D)
    return out.reshape(B, N, E)


# revision 19
# speedup vs baseline: 1.2707x; 1.2707x over previous
"""Fused OOQKV attention-with-generated-transform kernel for Trainium2.

Math (per head h):
  g = gelu(x @ Wg_h + bg_h)            # [T, 64, 64] per-token transform
  q,k,v = x @ W{q,k,v}_h + b           # [T, 64]
  qg[t] = q[t] @ g[t]
  att = softmax(qg @ k^T)              # per batch, no scaling
  out_h = att @ v

Sharding: head-parallel, 1 head per core (8 heads, 8 cores).

Implementation notes (v3):
- All projections are single-pass float16 matmuls (measured on HW: every
  matmul dtype streams 1 output column/cycle at 2.4 GHz, so f16's 11-bit
  mantissa is strictly better than bf16/f32r at identical cost; fp8
  DoubleRow only packs K=256 per instruction without halving per-column
  cost, so an error-compensated fp8 scheme loses to f16 1-pass).
- Biases enter as a K=1 ones-row matmul issued FIRST into each PSUM bank
  (start=True doubles as the bank reset).
- Wg columns are host-permuted e-major (col' = e*64+d) so gelu writes
  f16 contiguously and the qg d-reduction is innermost.
- qg on DVE: one f16 tensor_tensor multiply (4x/2x fast mode) against a
  broadcast q view, then a binary tree of f16 adds down to d=8 and a
  final tensor_reduce (tensor_reduce has no DVE fast mode, so the tree
  keeps most of the reduction in fast mode).
- Phase 2 runs after phase 1 (gelu and exp live in different ACT tables;
  interleaving them costs 1.3us per table reload). Within phase 2 the
  previous batch's AV matmuls are emitted mid-stream of the next batch's
  S matmuls, baseline-style, so PE/ACT pipeline across batches.
Host divides by the softmax denominator row (v is augmented with a ones
column initialized once on-chip) and transposes during the gather.
"""

import sys

sys.path.insert(0, "/opt/trn_rl_repo")

import numpy as np

B, N, E, H, D = 4, 1024, 512, 8, 64
T = B * N                 # 4096 flattened tokens
OC = 512                  # g output chunk (one PSUM bank)
NOC = (D * D) // OC       # 8 chunks per head
NKT = E // 128            # 4 contraction k-tiles
NTT = T // 128            # 32 token tiles
QVKW = 192                # fused q|v|k projection width
M = 8                     # cores
NMT = N // 128            # m tiles per batch
NNC = N // OC             # n chunks per batch

_cache = {}


def _build():
    if "nc" in _cache:
        return _cache["nc"]
    from contextlib import ExitStack

    import concourse.bass as bass
    import concourse.bacc as bacc
    import concourse.mybir as mybir
    import concourse.tile as tile
    from concourse.masks import make_identity

    F32 = mybir.dt.float32
    F32R = mybir.dt.float32r
    F16 = mybir.dt.float16
    AF = mybir.ActivationFunctionType
    ALU = mybir.AluOpType
    AX = mybir.AxisListType

    nc = bacc.Bacc(trn_type="TRN2")

    xT_d = nc.dram_tensor("xT16", [E, T], F16, kind="ExternalInput")
    Wg_d = nc.dram_tensor("Wg16", [E, D * D], F16, kind="ExternalInput")
    bg_d = nc.dram_tensor("bg16", [1, D * D], F16, kind="ExternalInput")
    Wq_d = nc.dram_tensor("Wqvk16", [E, QVKW], F16, kind="ExternalInput")
    bq_d = nc.dram_tensor("bqvk16", [1, QVKW], F16, kind="ExternalInput")
    outT_d = nc.dram_tensor("outT", [D + 1, T], F32, kind="ExternalOutput")

    with tile.TileContext(nc) as tc, ExitStack() as ctx:
        const = ctx.enter_context(tc.tile_pool(name="const", bufs=1))
        acts = ctx.enter_context(tc.tile_pool(name="acts", bufs=1))

        xT_sb, wg_sb, wq_sb = [], [], []
        for kt in range(NKT):
            xt = const.tile([128, T], F16, tag=f"x{kt}")
            nc.sync.dma_start(xt[:], xT_d[kt * 128:(kt + 1) * 128, :])
            xT_sb.append(xt)
            wt = const.tile([128, D * D], F16, tag=f"wg{kt}")
            nc.scalar.dma_start(wt[:], Wg_d[kt * 128:(kt + 1) * 128, :])
            wg_sb.append(wt)
            qt = const.tile([128, QVKW], F16, tag=f"wq{kt}")
            nc.scalar.dma_start(qt[:], Wq_d[kt * 128:(kt + 1) * 128, :])
            wq_sb.append(qt)
        bg_sb = const.tile([1, D * D], F16)
        nc.sync.dma_start(bg_sb[:], bg_d[:, :])
        bq_sb = const.tile([1, QVKW], F16)
        nc.sync.dma_start(bq_sb[:], bq_d[:, :])

        ones32 = const.tile([1, 128], F32)
        nc.gpsimd.memset(ones32[:], 1.0)
        ones16 = const.tile([1, 128], F16)
        nc.gpsimd.tensor_copy(ones16[:], ones32[:])
        onescol = const.tile([128, 1], F32)
        nc.gpsimd.memset(onescol[:], 1.0)
        ident = const.tile([128, 128], F16)
        make_identity(nc, ident[:])

        # persistent per-head activations
        v_sb = acts.tile([128, NTT, D + 1], F32R)   # v | ones column
        ov = onescol[:]
        ones_bc = bass.AP(tensor=ov.tensor, offset=ov.offset,
                          ap=[ov.ap[0], [0, NTT]])
        vv = v_sb[:]
        vcol = bass.AP(tensor=vv.tensor, offset=vv.offset + D,
                       ap=[vv.ap[0], [D + 1, NTT]])
        nc.vector.tensor_copy(vcol, ones_bc)
        kT_sb = acts.tile([D, T], F16)
        qgT_sb = acts.tile([D, T], F16)

        espool = ctx.enter_context(tc.tile_pool(name="es", bufs=14))
        outp = ctx.enter_context(tc.tile_pool(name="outp", bufs=2))

        p1 = ExitStack()
        pmain = p1.enter_context(
            tc.tile_pool(name="pmain", bufs=5, space="PSUM"))
        ptr = p1.enter_context(
            tc.tile_pool(name="ptr", bufs=1, space="PSUM"))
        gpool = p1.enter_context(tc.tile_pool(name="gp", bufs=2))
        dpool = p1.enter_context(tc.tile_pool(name="dp", bufs=2))
        tpool = p1.enter_context(tc.tile_pool(name="tp", bufs=1))

        prev_tr = []      # (tc0, k_nat, qg_t) awaiting PE transpose

        def emit_transposes():
            for tc0_, kn, qt in prev_tr:
                for src, dst in ((kn, kT_sb), (qt, qgT_sb)):
                    p_t = ptr.tile([D, 128], F16, tag="tr", name="tr")
                    nc.tensor.transpose(p_t[:], src[:], ident[:])
                    nc.vector.tensor_copy(dst[:, tc0_:tc0_ + 128], p_t[:])
            prev_tr.clear()

        # ---------------- phase 1: projections, g, qg ----------------
        for tt in range(NTT):
            tc0 = tt * 128
            g_buf = gpool.tile([128, NOC * OC], F16, tag="g")
            pq = None
            q_t = k_nat = None

            pairs = [("qvk", 0), (1, 2), (3, 4), (5, 6), (7,)]
            for pair in pairs:
                members = []
                for m_ in pair:
                    if m_ == "qvk":
                        pq = pmain.tile([128, OC], F32, tag="pg", name="pq")
                        members.append((m_, pq[:, 0:QVKW],
                                        bq_sb[:], 0, QVKW))
                    else:
                        pg = pmain.tile([128, OC], F32, tag="pg",
                                        name=f"pg{m_}")
                        members.append((m_, pg[:], bg_sb[:, m_ * OC:
                                                         (m_ + 1) * OC],
                                        m_ * OC, OC))
                # bias first: start=True resets the bank
                for m_, o, brow, c0_, w_ in members:
                    nc.tensor.matmul(o, ones16[:], brow,
                                     start=True, stop=False)
                for kt in range(NKT):
                    for m_, o, brow, c0_, w_ in members:
                        rhs = (wq_sb[kt][:] if m_ == "qvk"
                               else wg_sb[kt][:, c0_:c0_ + w_])
                        nc.tensor.matmul(
                            o, xT_sb[kt][:, tc0:tc0 + 128], rhs,
                            start=False, stop=(kt == NKT - 1))

                for m_, o, brow, c0_, w_ in members:
                    if m_ == "qvk":
                        q_t = dpool.tile([128, D], F16, tag="q")
                        nc.vector.tensor_copy(q_t[:], pq[:, 0:D])
                        k_nat = dpool.tile([128, D], F16, tag="k")
                        nc.vector.tensor_copy(k_nat[:], pq[:, 2 * D:3 * D])
                        nc.vector.tensor_copy(v_sb[:, tt, 0:D],
                                              pq[:, D:2 * D])
                    else:
                        nc.scalar.activation(
                            g_buf[:, m_ * OC:(m_ + 1) * OC], o, AF.Gelu)

            # qg contraction on DVE (all f16, fast mode)
            prod = dpool.tile([128, NOC * OC], F16, tag="prod")
            gv = g_buf[:]
            g3 = bass.AP(tensor=gv.tensor, offset=gv.offset,
                         ap=[gv.ap[0], [D, D], [1, D]])
            qv = q_t[:]
            q3 = bass.AP(tensor=qv.tensor, offset=qv.offset,
                         ap=[qv.ap[0], [0, D], [1, D]])
            pv = prod[:]
            p3 = bass.AP(tensor=pv.tensor, offset=pv.offset,
                         ap=[pv.ap[0], [D, D], [1, D]])
            nc.vector.tensor_tensor(p3, g3, q3, op=ALU.mult)

            tre1 = tpool.tile([128, D * 32], F16, tag="t1")
            tre2 = tpool.tile([128, D * 16], F16, tag="t2")
            tre3 = tpool.tile([128, D * 8], F16, tag="t3")
            lvl_in, w_ = prod, 64
            for tre in (tre1, tre2, tre3):
                h_ = w_ // 2
                iv = lvl_in[:]
                a0 = bass.AP(tensor=iv.tensor, offset=iv.offset,
                             ap=[iv.ap[0], [w_, D], [1, h_]])
                a1 = bass.AP(tensor=iv.tensor, offset=iv.offset + h_,
                             ap=[iv.ap[0], [w_, D], [1, h_]])
                ov_ = tre[:]
                o3 = bass.AP(tensor=ov_.tensor, offset=ov_.offset,
                             ap=[ov_.ap[0], [h_, D], [1, h_]])
                nc.vector.tensor_tensor(o3, a0, a1, op=ALU.add)
                lvl_in, w_ = tre, h_
            qg_t = dpool.tile([128, D], F16, tag="qg")
            t3v = tre3[:]
            t33 = bass.AP(tensor=t3v.tensor, offset=t3v.offset,
                          ap=[t3v.ap[0], [8, D], [1, 8]])
            with nc.allow_low_precision(reason="f16 qg partials"):
                nc.vector.tensor_reduce(qg_t[:], t33, axis=AX.X, op=ALU.add)

            emit_transposes()
            prev_tr.append((tc0, k_nat, qg_t))
        emit_transposes()

        # ---------------- phase 2: attention ----------------
        p1.close()
        p2 = ExitStack()
        spair = p2.enter_context(
            tc.tile_pool(name="spair", bufs=3, space="PSUM"))
        pav = p2.enter_context(
            tc.tile_pool(name="pav", bufs=2, space="PSUM"))

        pending_av = []   # (b, es-pair list) awaiting AV emission

        def emit_av():
            if not pending_av:
                return
            for bb, pairs_ in pending_av:
                for nch in range(NNC):
                    pv_ = pav.tile([D + 1, OC], F32, tag="av",
                                   name=f"pav{nch}")
                    for mt in range(NMT):
                        nc.tensor.matmul(
                            pv_[:], v_sb[:, bb * NMT + mt, :],
                            pairs_[mt][:, nch * OC:(nch + 1) * OC],
                            start=(mt == 0), stop=(mt == NMT - 1))
                    nc0 = bb * N + nch * OC
                    o_t = outp.tile([D + 1, OC], F32, tag="o", name="o_t")
                    nc.vector.tensor_copy(o_t[:], pv_[:])
                    nc.sync.dma_start(outT_d[:, nc0:nc0 + OC], o_t[:])
            pending_av.clear()

        for b in range(B):
            bpairs = []
            for mt in range(NMT):
                if mt == 2:
                    emit_av()     # prior batch's AV, mid-stream
                mc0 = b * N + mt * 128
                sp = spair.tile([128, 2 * OC], F32, tag="sp", name="sp")
                for nch in range(NNC):
                    nc0 = b * N + nch * OC
                    nc.tensor.matmul(sp[:, nch * OC:(nch + 1) * OC],
                                     kT_sb[:, mc0:mc0 + 128],
                                     qgT_sb[:, nc0:nc0 + OC],
                                     start=True, stop=True)
                e_t = espool.tile([128, 2 * OC], F32R, tag="es")
                nc.scalar.activation(e_t[:], sp[:], AF.Exp)
                bpairs.append(e_t)
            pending_av.append((b, bpairs))
        emit_av()
        p2.close()

    nc.compile()
    _cache["nc"] = nc
    return nc


def _make_in_maps(x, Wq, bq, Wk, bk, Wv, bv, Wg, bg):
    import ml_dtypes  # noqa: F401
    F16 = np.float16
    xT16 = np.ascontiguousarray(
        np.asarray(x, np.float32).reshape(T, E).T).astype(F16)

    # e-major permutation for Wg columns within each head
    new_e, new_d = np.meshgrid(np.arange(D), np.arange(D), indexing="ij")
    old_of_new = (new_d * D + new_e).reshape(-1)

    in_maps = []
    for h in range(M):
        c0 = h * D
        g0 = h * D * D
        Wp = np.asarray(Wg, np.float32)[:, g0:g0 + D * D][:, old_of_new]
        bgp = np.asarray(bg, np.float32)[g0:g0 + D * D][old_of_new]

        QW = np.zeros((E, QVKW), np.float32)
        QW[:, 0:D] = Wq[:, c0:c0 + D]
        QW[:, D:2 * D] = Wv[:, c0:c0 + D]
        QW[:, 2 * D:3 * D] = Wk[:, c0:c0 + D]
        qb = np.zeros((1, QVKW), np.float32)
        qb[0, 0:D] = bq[c0:c0 + D]
        qb[0, D:2 * D] = bv[c0:c0 + D]
        qb[0, 2 * D:3 * D] = bk[c0:c0 + D]

        in_maps.append(dict(
            xT16=xT16,
            Wg16=np.ascontiguousarray(Wp).astype(F16),
            bg16=bgp.reshape(1, D * D).astype(F16),
            Wqvk16=np.ascontiguousarray(QW).astype(F16),
            bqvk16=qb.astype(F16),
        ))
    return in_maps


def kernel(x, Wq, bq, Wk, bk, Wv, bv, Wg, bg):
    from concourse import bass_utils

    nc = _build()
    in_maps = _make_in_maps(x, Wq, bq, Wk, bk, Wv, bv, Wg, bg)
    res = bass_utils.run_bass_kernel_spmd(nc, in_maps, core_ids=list(range(M)))
    out = np.empty((B, N, H, D), dtype=np.float32)
    for h in range(M):
        oT = res.results[h]["outT"]           # [65, T]
        o = (oT[:D] / oT[D:D + 1]).T          # [T, 64]
        out[:, :, h, :] = o.reshape(B, N, D)
    return out.reshape(B, N, E)
